# revision 30
# baseline (speedup 1.0000x reference)
"""Single-head attention on 8 Trainium2 NeuronCores.

Problem: B=8, S=2048, WIDTH=1024, HEAD=64 single attention head.
Sharding: data-parallel over batch -- batch b runs on core b. No collectives.

v2: position-block pipelined, bf16 datapath.

x^T is host-prepped to bf16 and loaded in 8 column blocks of 256
positions (0.5 MB each).  As each block lands, K^T/Q^T (stacked, with
Wq pre-scaled) and V^T are projected for those positions, copied out
with biases on DVE, partition-remapped (Q^T down to 0:64, K^T up to
64:128) via DMA, and V^T chunks PE-transposed into V' (+ones column).
Attention units (k-chunk x query-half) are emitted interleaved with the
block loop as soon as their inputs exist, so the ACT exp chain (the
critical 33us of work) starts ~4us in, overlapping the remaining x DMA
and projections instead of following them.

Per unit (qh, k): scores^T = K^T_k.T @ Q^T[qh] ([128,1024] PSUM, even k
on PE row-half A, odd k row-half B via tile_position -- concurrent);
es = exp(scores^T + mask_bias) on ACT (bf16 out); O'^T[qh] += V'_k.T @ es
(PE, accumulating [65, 1024] PSUM; row 64 = softmax denominators from
the ones column).  O-tail per 128-query tile: bf16 staging copy,
PE-transpose, DVE reciprocal + scale, DMA out.
"""

import os
from contextlib import ExitStack

import numpy as np

import concourse.bass as bass
import concourse.tile as tile
from concourse import mybir
from concourse.bass import ts

S = 2048
W = 1024
H = 64
N_CORES = 8
WC = W // 128   # 8 w-chunks
KC = S // 128   # 16 k-chunks
NB = 8          # x position blocks
BP = S // NB    # 256 positions per block
QB = 1024       # query-half size
QT = QB // 128  # 8 q-tiles per half

F32 = mybir.dt.float32
BF16 = mybir.dt.bfloat16
AF = mybir.ActivationFunctionType

# score/exp/PV work is emitted as pairs (k-chunks kk, kk+1 on PE row
# halves A/B, back-to-back so they overlap) over a query range.  Early
# pairs use 512-wide query granules (qg = index in units of 512) so the
# ACT exp chain starts ~4.5us in, while x blocks 2..7 are still loading;
# once queries 0:1024 exist, pairs go 1024-wide (qh units).
# PRE pairs depend only on OLDER blocks and are emitted before the
# section's projection matmuls, so the exp chain never waits on proj.
PRE_SCHED = {
    1: [("h", 0, 0)],
    2: [("g", 0, 2)],
    4: [("p", 0, 6)],
    5: [("p", 0, 8)],
    6: [("p", 0, 10)],
    7: [("p", 0, 12)],
}
POST_SCHED = {
    1: [("h", 1, 0)],
    2: [("g", 0, 4)],
    3: [("g", 1, 0), ("g", 1, 2), ("g", 1, 4)],
}


def _emit(ctx, tc, xb, wkq, wv, bkq, bv, identb, ident, mbias, vones, zpad,
          out, rep="", probe=None):
    nc = tc.nc

    def pool(name, **kw):
        return ctx.enter_context(tc.tile_pool(name=name + rep, **kw))

    singles = pool("singles", bufs=1)
    wkq_sb = singles.tile([128, WC * 128], BF16)
    nc.scalar.dma_start(out=wkq_sb, in_=wkq)
    wv_sb = singles.tile([128, WC * H], BF16)
    nc.scalar.dma_start(out=wv_sb, in_=wv)
    bkq_sb = singles.tile([128, 1], F32)
    nc.scalar.dma_start(out=bkq_sb, in_=bkq)
    bvb_sb = singles.tile([128, H], BF16)
    nc.scalar.dma_start(out=bvb_sb, in_=bv)
    identb_sb = singles.tile([128, H], BF16)
    nc.scalar.dma_start(out=identb_sb, in_=identb)
    ident_sb = singles.tile([128, 128], BF16)
    nc.scalar.dma_start(out=ident_sb, in_=ident)
    mbias_sb = singles.tile([128, KC], F32)
    nc.scalar.dma_start(out=mbias_sb, in_=mbias)
    warm_sb = singles.tile([1, 1], F32)
    nc.scalar.activation(warm_sb, mbias_sb[0:1, 0:1], AF.Exp,
                         bias=mbias_sb[0:1, 0:1], scale=1.0)

    kq_sb = singles.tile([128, S], BF16)   # rows 0:64 K^T, rows 64:128 Q^T
    q_sb = singles.tile([64, S], BF16)     # Q^T at partitions 0:64
    k2_sb = singles.tile([128, S], BF16)   # K^T replicated at partitions 64:128
    vT_sb = singles.tile([64, S], BF16)
    v_sb = singles.tile([128, KC, H + 1], BF16)  # V' chunks (+ones col)
    oTs_sb = singles.tile([128, 2, QB], BF16)    # O^T staging, rows 65:128 zero
    nc.gpsimd.dma_start(
        out=v_sb[:, :, H : H + 1],
        in_=vones.rearrange("p (k one) -> p k one", one=1),
    )
    nc.gpsimd.dma_start(
        out=oTs_sb[H + 1 : 128, :, :],
        in_=zpad.rearrange("p (a b) -> p a b", a=2),
    )

    # pools
    xp = pool("xp", bufs=3)
    kqvps = pool("kqvps", bufs=1, space="PSUM")
    scps = pool("scps", bufs=2, space="PSUM")
    otps = pool("otps", bufs=1, space="PSUM")
    trps = pool("trps", bufs=1, space="PSUM")
    es_pool = pool("es", bufs=6)
    rec_pool = pool("rec", bufs=4)
    ob_pool = pool("ob", bufs=2)

    out_v = out.rearrange("(qh t p) h -> qh p t h", p=128, t=QT)

    oT_ps = [None, None]
    ob_sb = [None, None]
    pending = []   # (q0, qw, k, es) awaiting their PV matmul
    started = set()  # (qh, h) oT 512-regions already start=True'd

    def flush_pv():
        for q0, qw, k, es in pending:
            qh = q0 // QB
            if oT_ps[qh] is None:
                ot_tile = otps.tile([H + 1, QB], F32, tag="otps" + rep)
                oT_ps[qh] = ot_tile
            for h in range(qw // 512):
                hreg = (q0 % QB) // 512 + h
                st = (qh, hreg) not in started
                started.add((qh, hreg))
                nc.tensor.matmul(
                    oT_ps[qh][:, ts(hreg, 512)], v_sb[:, k, :],
                    es[:, ts(h, 512)], start=st, stop=(k == KC - 1),
                )
        pending.clear()

    def emit_pair(q0, qw, kk, flush=True):
        sc0 = scps.tile([128, qw], F32, tag="scps" + rep)
        sc1 = scps.tile([128, qw], F32, tag="scps" + rep)
        step = min(qw, 512)
        for h in range(qw // step):
            hs = slice(q0 + h * step, q0 + (h + 1) * step)
            nc.tensor.matmul(sc0[:, ts(h, step)], kq_sb[0:64, ts(kk, 128)],
                             q_sb[:, hs])
            nc.tensor.matmul(
                sc1[:, ts(h, step)], k2_sb[64:128, ts(kk + 1, 128)],
                kq_sb[64:128, hs], tile_position=(64, 0),
            )
        if flush and probe != "nopv":
            flush_pv()
        for k, sc in ((kk, sc0), (kk + 1, sc1)):
            es = es_pool.tile([128, qw], BF16)
            nc.scalar.activation(
                es, sc, AF.Exp, bias=mbias_sb[:, k : k + 1], scale=1.0
            )
            if probe != "nopv":
                pending.append((q0, qw, k, es))

    h_es = {}

    def emit_half_pair(half, kk):
        # scores+exp for k-chunks kk,kk+1 on queries half*256:(half+1)*256,
        # filling half of a shared 512-wide es tile; PV deferred to half 1
        hs = slice(half * 256, (half + 1) * 256)
        scs = []
        for i, k in enumerate((kk, kk + 1)):
            sc = scps.tile([128, 256], F32, tag="scps" + rep)
            if k % 2 == 1:
                nc.tensor.matmul(
                    sc, k2_sb[64:128, ts(k, 128)], kq_sb[64:128, hs],
                    tile_position=(64, 0),
                )
            else:
                nc.tensor.matmul(sc, kq_sb[0:64, ts(k, 128)], q_sb[:, hs])
            scs.append(sc)
        for i, k in enumerate((kk, kk + 1)):
            if (k, "es") not in h_es:
                es_h = es_pool.tile([128, 512], BF16, tag="esh")
                h_es[(k, "es")] = es_h
            es = h_es[(k, "es")]
            nc.scalar.activation(
                es[:, hs], scs[i], AF.Exp,
                bias=mbias_sb[:, k : k + 1], scale=1.0,
            )
            if half == 1:
                pending.append((0, 512, k, es))

    def emit_sched(kind, qidx, kk):
        if kind == "h":
            emit_half_pair(qidx, kk)
        elif kind == "g":
            emit_pair(qidx * 512, 512, kk)
        else:
            emit_pair(qidx * QB, QB, kk)

    def emit_otail_copies(qh):
        for h in range(2):
            nc.vector.tensor_copy(
                oTs_sb[0 : H + 1, qh, ts(h, 512)], oT_ps[qh][:, ts(h, 512)]
            )

    def emit_otail_rest(qh, t, ps_pool):
        if ob_sb[qh] is None:
            ob_tile = ob_pool.tile([128, QT, H], F32, tag="ob" + rep)
            ob_sb[qh] = ob_tile
        op = ps_pool.tile([128, 128], BF16, tag=ps_pool.name)
        nc.tensor.transpose(op, oTs_sb[:, qh, ts(t, 128)], ident_sb)
        rec = rec_pool.tile([128, 1], F32)
        nc.vector.reciprocal(rec, op[:, H : H + 1])
        nc.vector.tensor_scalar_mul(ob_sb[qh][:, t, :], op[:, 0:H], rec)
        if t % 4 == 3:
            dma_eng = nc.scalar if (qh == 1 and t == QT - 1) else nc.sync
            dma_eng.dma_start(
                out=out_v[qh][:, t - 3 : t + 1, :],
                in_=ob_sb[qh][:, t - 3 : t + 1, :],
            )

    # ---------------- pipelined block loop ----------------
    xts = {}
    for j in range(NB):
        for kind, qidx, kk in PRE_SCHED.get(j, ()) if probe != "proj" else ():
            emit_sched(kind, qidx, kk)
        if j in xts:
            xt = xts.pop(j)
        else:
            xt = xp.tile([128, WC, BP], BF16)
            nc.sync.dma_start(
                out=xt, in_=xb[j].rearrange("p (g t) -> p g t", g=WC)
            )
        if j == 3:
            for jj in range(4, NB):
                xt2 = xp.tile([128, WC, BP], BF16, tag="xt2")
                nc.sync.dma_start(
                    out=xt2, in_=xb[jj].rearrange("p (g t) -> p g t", g=WC)
                )
                xts[jj] = xt2
        pj_ps = kqvps.tile([128, 2 * BP], F32, tag="kqvps" + rep)
        kq_ps = pj_ps[:, 0:BP]
        vT_ps = pj_ps[0:64, BP : BP + BP]
        for g in range(WC):
            nc.tensor.matmul(
                kq_ps, wkq_sb[:, ts(g, 128)], xt[:, g, :],
                start=(g == 0), stop=(g == WC - 1),
            )
        blk = slice(j * BP, (j + 1) * BP)
        nc.vector.tensor_scalar_add(kq_sb[:, blk], kq_ps, bkq_sb)
        rm_eng = nc.scalar if j <= 3 else nc.sync
        rm_eng.dma_start(out=q_sb[:, blk], in_=kq_sb[64:128, blk])
        rm_eng.dma_start(out=k2_sb[64:128, blk], in_=kq_sb[0:64, blk])
        for g in range(WC):
            nc.tensor.matmul(
                vT_ps, wv_sb[:, ts(g, H)], xt[:, g, :],
                start=(g == 0), stop=(g == WC - 1),
            )
        nc.vector.tensor_copy(vT_sb[0:64, blk], vT_ps)
        for kk in range(2 * j, 2 * j + 2):
            vp = trps.tile([128, H], BF16, tag="trps" + rep)
            nc.tensor.transpose(
                vp, vT_sb[:, ts(kk, 128)], ident_sb[0:64, 0:64]
            )
            nc.vector.scalar_tensor_tensor(
                v_sb[:, kk, 0:H], vp, 1.0, bvb_sb,
                mybir.AluOpType.mult, mybir.AluOpType.add,
            )
        if probe == "proj":
            continue
        for kind, qidx, kk in POST_SCHED.get(j, ()):
            emit_sched(kind, qidx, kk)

    if probe in ("proj", "nopv"):
        if probe == "nopv":
            for kind, qidx, kk in [("p", 0, 6), ("p", 0, 8), ("p", 0, 10),
                                   ("p", 0, 12), ("p", 0, 14)] +                                   [("p", 1, kk) for kk in range(0, KC, 2)]:
                pass
        nc.sync.dma_start(
            out=out.rearrange("(a p) h -> p a h", p=128),
            in_=kq_sb.bitcast(F32).rearrange("p (a h) -> p a h", h=H),
        )
        return

    # qh1 pairs; qh0's O^T must vacate the single otps buffer before
    # PV(1,0), so its staging copies are emitted (DVE) before that flush
    emit_pair(0, QB, 14)       # last qh0 pair
    emit_pair(QB, QB, 0)       # flushes PV(0,14), PV(0,15)
    emit_pair(QB, QB, 2, flush=False)
    emit_otail_copies(0)       # frees oT_ps[0]
    for kk in range(4, KC, 2):
        emit_pair(QB, QB, kk)
        if kk <= 10:
            i = (kk - 4) // 2
            emit_otail_rest(0, 2 * i, trps)
            emit_otail_rest(0, 2 * i + 1, trps)
    # final pair region-major: all PVs + staging copies first (so the PE
    # never stalls behind a DVE copy), then the per-tile rests
    for h in range(2):
        for q0, qw, k, es in pending:
            nc.tensor.matmul(
                oT_ps[1][:, ts(h, 512)], v_sb[:, k, :], es[:, ts(h, 512)],
                start=False, stop=(k == KC - 1),
            )
        nc.vector.tensor_copy(
            oTs_sb[0 : H + 1, 1, ts(h, 512)], oT_ps[1][:, ts(h, 512)]
        )
    for t in range(QT):
        emit_otail_rest(1, t, scps)
    pending.clear()


def split_multi_waits(nc):
    """This walrus build encodes at most ONE sync-wait per hw instruction.
    Hoist all but the last wait of any multi-wait instruction into standalone
    single-wait NoOps on the same engine queue (semantically identical:
    engine-queue execution is in-order)."""
    import bass_rust

    ctr = 0
    for blk in nc.m.functions[0].blocks:
        insts = blk.instructions
        out = []
        changed = False
        for inst in insts:
            si = inst.sync_info
            if si is not None and si.on_wait and len(si.on_wait) > 1:
                waits = list(si.on_wait)
                for w in waits[:-1]:
                    ctr += 1
                    nop = mybir.InstNoOp(name=f"WSPLIT-{ctr}", ins=[], outs=[])
                    nop.engine = inst.engine
                    nop.sync_info = bass_rust.SyncInfo(on_wait=[w], on_update=[])
                    out.append(nop)
                inst.sync_info = bass_rust.SyncInfo(
                    on_wait=[waits[-1]], on_update=list(si.on_update or [])
                )
                out.append(inst)
                changed = True
            else:
                out.append(inst)
        if changed:
            insts[:] = out
    return nc


def build_bass(split=True, repeat=1, probe=None, **_):
    nc = bass.Bass("TRN2", target_bir_lowering=False, debug=False)
    xbt = nc.dram_tensor("xb", [NB, 128, WC * BP], BF16, kind="ExternalInput").ap()
    wkq = nc.dram_tensor("wkq", [128, WC * 128], BF16, kind="ExternalInput").ap()
    wv = nc.dram_tensor("wv", [128, WC * H], BF16, kind="ExternalInput").ap()
    bkq = nc.dram_tensor("bkq", [128, 1], F32, kind="ExternalInput").ap()
    bv = nc.dram_tensor("bv", [128, H], BF16, kind="ExternalInput").ap()
    identb = nc.dram_tensor("identb", [128, H], BF16, kind="ExternalInput").ap()
    ident = nc.dram_tensor("ident", [128, 128], BF16, kind="ExternalInput").ap()
    mbias = nc.dram_tensor("mbias", [128, KC], F32, kind="ExternalInput").ap()
    vones = nc.dram_tensor("vones", [128, KC], BF16, kind="ExternalInput").ap()
    zpad = nc.dram_tensor("zpad", [128 - H - 1, 2 * QB], BF16,
                          kind="ExternalInput").ap()
    out = nc.dram_tensor("out", [S, H], F32, kind="ExternalOutput").ap()
    with tile.TileContext(nc) as tc:
        for r in range(repeat):
            with ExitStack() as ctx:
                _emit(
                    ctx, tc, xbt, wkq, wv, bkq, bv, identb, ident, mbias,
                    vones, zpad, out, rep=(f"_r{r}" if r else ""), probe=probe,
                )
    if split:
        split_multi_waits(nc)
    return nc


def prep_in_maps(x, attn_mask, Wq, bq, Wk, bk, Wv, bv):
    import ml_dtypes

    bf = ml_dtypes.bfloat16
    x = np.asarray(x, dtype=np.float32)
    attn_mask = np.asarray(attn_mask)
    Wq = np.asarray(Wq, dtype=np.float32)
    Wk = np.asarray(Wk, dtype=np.float32)
    Wv = np.asarray(Wv, dtype=np.float32)
    bq = np.asarray(bq, dtype=np.float32)
    bk = np.asarray(bk, dtype=np.float32)
    bv = np.asarray(bv, dtype=np.float32)

    scale = np.float32(H) ** np.float32(-0.5)
    # [Wk | Wq*scale] -> per-w-chunk stationary layout [128, WC*128]
    wkq = np.concatenate([Wk, Wq * scale], axis=1)  # [W, 128]
    wkq = np.ascontiguousarray(
        wkq.reshape(WC, 128, 128).transpose(1, 0, 2).reshape(128, WC * 128)
    ).astype(bf)
    wv_h = np.ascontiguousarray(
        Wv.reshape(WC, 128, H).transpose(1, 0, 2).reshape(128, WC * H)
    ).astype(bf)
    bkq = np.concatenate([bk, bq * scale]).reshape(128, 1)
    bv_h = np.broadcast_to(bv.reshape(1, H), (128, H)).astype(bf)
    ident = np.eye(128, dtype=np.float32).astype(bf)
    identb = np.ascontiguousarray(
        np.concatenate([np.eye(H), np.eye(H)], axis=0).astype(bf)
    )

    in_maps = []
    for c in range(N_CORES):
        # xb[j, p, g*BP+t] = x[c]^T[g*128+p, j*BP+t]
        xT_c = x[c].T.astype(bf)  # [W, S]
        xb = np.ascontiguousarray(
            xT_c.reshape(WC, 128, NB, BP).transpose(2, 1, 0, 3)
            .reshape(NB, 128, WC * BP)
        )
        m = attn_mask[c].astype(np.float32)  # [S]
        mb = np.where(m != 0, np.float32(0.0), np.float32(-1e30))
        mbias = np.ascontiguousarray(mb.reshape(KC, 128).T)  # [128, KC]
        in_maps.append(
            {
                "xb": xb,
                "wkq": wkq,
                "wv": wv_h,
                "bkq": np.ascontiguousarray(bkq),
                "bv": np.ascontiguousarray(bv_h),
                "identb": identb,
                "ident": ident,
                "mbias": mbias,
                "vones": np.ones((128, KC), dtype=np.float32).astype(bf),
                "zpad": np.zeros((128 - H - 1, 2 * QB), dtype=np.float32).astype(bf),
            }
        )
    return in_maps


def run(x, attn_mask, Wq, bq, Wk, bk, Wv, bv, trace=False, **rb_kwargs):
    from concourse.bass_utils import run_bass_kernel_spmd

    nc = build_bass()
    in_maps = prep_in_maps(x, attn_mask, Wq, bq, Wk, bk, Wv, bv)
    res = run_bass_kernel_spmd(
        nc, in_maps, core_ids=list(range(N_CORES)), trace=trace, **rb_kwargs
    )
    out = np.stack([r["out"] for r in res.results]).astype(np.float32)
    return out, res


def kernel(x, attn_mask, Wq, bq, Wk, bk, Wv, bv):
    out, _ = run(x, attn_mask, Wq, bq, Wk, bk, Wv, bv, trace=False)
    return out


# revision 35
# speedup vs baseline: 2.0225x; 2.0225x over previous
"""Single-head attention on 8 Trainium2 NeuronCores.

Problem: B=8, S=2048, WIDTH=1024, HEAD=64 single attention head.
Sharding: data-parallel over batch -- batch b runs on core b. No collectives.

v2: position-block pipelined, bf16 datapath.

x^T is host-prepped to bf16 and loaded in 8 column blocks of 256
positions (0.5 MB each).  As each block lands, K^T/Q^T (stacked, with
Wq pre-scaled) and V^T are projected for those positions, copied out
with biases on DVE, partition-remapped (Q^T down to 0:64, K^T up to
64:128) via DMA, and V^T chunks PE-transposed into V' (+ones column).
Attention units (k-chunk x query-half) are emitted interleaved with the
block loop as soon as their inputs exist, so the ACT exp chain (the
critical 33us of work) starts ~4us in, overlapping the remaining x DMA
and projections instead of following them.

Per unit (qh, k): scores^T = K^T_k.T @ Q^T[qh] ([128,1024] PSUM, even k
on PE row-half A, odd k row-half B via tile_position -- concurrent);
es = exp(scores^T + mask_bias) on ACT (bf16 out); O'^T[qh] += V'_k.T @ es
(PE, accumulating [65, 1024] PSUM; row 64 = softmax denominators from
the ones column).  O-tail per 128-query tile: bf16 staging copy,
PE-transpose, DVE reciprocal + scale, DMA out.
"""

import os
from contextlib import ExitStack

import numpy as np

import concourse.bass as bass
import concourse.tile as tile
from concourse import mybir
from concourse.bass import ts

S = 2048
W = 1024
H = 64
N_CORES = 8
WC = W // 128   # 8 w-chunks
KC = S // 128   # 16 k-chunks
NB = 8          # x position blocks
BP = S // NB    # 256 positions per block
QB = 1024       # query-half size
QT = QB // 128  # 8 q-tiles per half

F32 = mybir.dt.float32
BF16 = mybir.dt.bfloat16
AF = mybir.ActivationFunctionType

# score/exp/PV work is emitted as pairs (k-chunks kk, kk+1 on PE row
# halves A/B, back-to-back so they overlap) over a query range.  Early
# pairs use 512-wide query granules (qg = index in units of 512) so the
# ACT exp chain starts ~4.5us in, while x blocks 2..7 are still loading;
# once queries 0:1024 exist, pairs go 1024-wide (qh units).
# PRE pairs depend only on OLDER blocks and are emitted before the
# section's projection matmuls, so the exp chain never waits on proj.
PRE_SCHED = {
    1: [("h", 0, 0)],
    2: [("g", 0, 2)],
    4: [("p", 0, 6)],
    5: [("p", 0, 8)],
    6: [("p", 0, 10)],
    7: [("p", 0, 12)],
}
POST_SCHED = {
    1: [("h", 1, 0)],
    2: [("g", 0, 4)],
    3: [("g", 1, 0), ("g", 1, 2), ("g", 1, 4)],
}


def _emit(ctx, tc, xb, wkq, wv, bkq, bv, identb, ident, mbias, vones, zpad,
          out, rep="", probe=None):
    nc = tc.nc

    def pool(name, **kw):
        return ctx.enter_context(tc.tile_pool(name=name + rep, **kw))

    singles = pool("singles", bufs=1)
    wkq_sb = singles.tile([128, WC * 128], BF16)
    nc.scalar.dma_start(out=wkq_sb, in_=wkq)
    wv_sb = singles.tile([128, WC * H], BF16)
    nc.scalar.dma_start(out=wv_sb, in_=wv)
    bkq_sb = singles.tile([128, 1], F32)
    nc.scalar.dma_start(out=bkq_sb, in_=bkq)
    bvb_sb = singles.tile([128, H], BF16)
    nc.scalar.dma_start(out=bvb_sb, in_=bv)
    identb_sb = singles.tile([128, H], BF16)
    nc.scalar.dma_start(out=identb_sb, in_=identb)
    ident_sb = singles.tile([128, 128], BF16)
    nc.scalar.dma_start(out=ident_sb, in_=ident)
    mbias_sb = singles.tile([128, KC], F32)
    nc.scalar.dma_start(out=mbias_sb, in_=mbias)
    warm_sb = singles.tile([1, 1], F32)
    nc.scalar.activation(warm_sb, mbias_sb[0:1, 0:1], AF.Exp,
                         bias=mbias_sb[0:1, 0:1], scale=1.0)

    kq_sb = singles.tile([128, S], BF16)   # rows 0:64 K^T, rows 64:128 Q^T
    q_sb = singles.tile([64, S], BF16)     # Q^T at partitions 0:64
    k2_sb = singles.tile([128, S], BF16)   # K^T replicated at partitions 64:128
    vT_sb = singles.tile([64, S], BF16)
    v_sb = singles.tile([128, KC, H + 1], BF16)  # V' chunks (+ones col)
    oTs_sb = singles.tile([128, 2, QB], BF16)    # O^T staging, rows 65:128 zero
    nc.gpsimd.dma_start(
        out=v_sb[:, :, H : H + 1],
        in_=vones.rearrange("p (k one) -> p k one", one=1),
    )
    nc.gpsimd.dma_start(
        out=oTs_sb[H + 1 : 128, :, :],
        in_=zpad.rearrange("p (a b) -> p a b", a=2),
    )

    # pools
    xp = pool("xp", bufs=3)
    kqvps = pool("kqvps", bufs=1, space="PSUM")
    scps = pool("scps", bufs=2, space="PSUM")
    otps = pool("otps", bufs=1, space="PSUM")
    trps = pool("trps", bufs=1, space="PSUM")
    es_pool = pool("es", bufs=6)
    rec_pool = pool("rec", bufs=4)
    ob_pool = pool("ob", bufs=2)

    out_v = out.rearrange("(qh t p) h -> qh p t h", p=128, t=QT)

    oT_ps = [None, None]
    ob_sb = [None, None]
    pending = []   # (q0, qw, k, es) awaiting their PV matmul
    started = set()  # (qh, h) oT 512-regions already start=True'd

    def flush_pv():
        for q0, qw, k, es in pending:
            qh = q0 // QB
            if oT_ps[qh] is None:
                ot_tile = otps.tile([H + 1, QB], F32, tag="otps" + rep)
                oT_ps[qh] = ot_tile
            for h in range(qw // 512):
                hreg = (q0 % QB) // 512 + h
                st = (qh, hreg) not in started
                started.add((qh, hreg))
                nc.tensor.matmul(
                    oT_ps[qh][:, ts(hreg, 512)], v_sb[:, k, :],
                    es[:, ts(h, 512)], start=st, stop=(k == KC - 1),
                )
        pending.clear()

    def emit_pair(q0, qw, kk, flush=True):
        sc0 = scps.tile([128, qw], F32, tag="scps" + rep)
        sc1 = scps.tile([128, qw], F32, tag="scps" + rep)
        step = min(qw, 512)
        for h in range(qw // step):
            hs = slice(q0 + h * step, q0 + (h + 1) * step)
            nc.tensor.matmul(sc0[:, ts(h, step)], kq_sb[0:64, ts(kk, 128)],
                             q_sb[:, hs])
            nc.tensor.matmul(
                sc1[:, ts(h, step)], k2_sb[64:128, ts(kk + 1, 128)],
                kq_sb[64:128, hs], tile_position=(64, 0),
            )
        if flush and probe != "nopv":
            flush_pv()
        for k, sc in ((kk, sc0), (kk + 1, sc1)):
            es = es_pool.tile([128, qw], BF16)
            nc.scalar.activation(
                es, sc, AF.Exp, bias=mbias_sb[:, k : k + 1], scale=1.0
            )
            if probe != "nopv":
                pending.append((q0, qw, k, es))

    h_es = {}

    def emit_half_pair(half, kk):
        # scores+exp for k-chunks kk,kk+1 on queries half*256:(half+1)*256,
        # filling half of a shared 512-wide es tile; PV deferred to half 1
        hs = slice(half * 256, (half + 1) * 256)
        scs = []
        for i, k in enumerate((kk, kk + 1)):
            sc = scps.tile([128, 256], F32, tag="scps" + rep)
            if k % 2 == 1:
                nc.tensor.matmul(
                    sc, k2_sb[64:128, ts(k, 128)], kq_sb[64:128, hs],
                    tile_position=(64, 0),
                )
            else:
                nc.tensor.matmul(sc, kq_sb[0:64, ts(k, 128)], q_sb[:, hs])
            scs.append(sc)
        for i, k in enumerate((kk, kk + 1)):
            if (k, "es") not in h_es:
                es_h = es_pool.tile([128, 512], BF16, tag="esh")
                h_es[(k, "es")] = es_h
            es = h_es[(k, "es")]
            nc.scalar.activation(
                es[:, hs], scs[i], AF.Exp,
                bias=mbias_sb[:, k : k + 1], scale=1.0,
            )
            if half == 1:
                pending.append((0, 512, k, es))

    def emit_sched(kind, qidx, kk):
        if kind == "h":
            emit_half_pair(qidx, kk)
        elif kind == "g":
            emit_pair(qidx * 512, 512, kk)
        else:
            emit_pair(qidx * QB, QB, kk)

    def emit_otail_copies(qh):
        for h in range(2):
            nc.vector.tensor_copy(
                oTs_sb[0 : H + 1, qh, ts(h, 512)], oT_ps[qh][:, ts(h, 512)]
            )

    def emit_otail_rest(qh, t, ps_pool):
        if ob_sb[qh] is None:
            ob_tile = ob_pool.tile([128, QT, H], F32, tag="ob" + rep)
            ob_sb[qh] = ob_tile
        op = ps_pool.tile([128, 128], BF16, tag=ps_pool.name)
        nc.tensor.transpose(op, oTs_sb[:, qh, ts(t, 128)], ident_sb)
        rec = rec_pool.tile([128, 1], F32)
        nc.vector.reciprocal(rec, op[:, H : H + 1])
        nc.vector.tensor_scalar_mul(ob_sb[qh][:, t, :], op[:, 0:H], rec)
        if t % 4 == 3:
            dma_eng = nc.scalar if (qh == 1 and t == QT - 1) else nc.sync
            dma_eng.dma_start(
                out=out_v[qh][:, t - 3 : t + 1, :],
                in_=ob_sb[qh][:, t - 3 : t + 1, :],
            )

    # ---------------- pipelined block loop ----------------
    xts = {}
    for j in range(NB):
        for kind, qidx, kk in PRE_SCHED.get(j, ()) if probe != "proj" else ():
            emit_sched(kind, qidx, kk)
        if j in xts:
            xt = xts.pop(j)
        else:
            xt = xp.tile([128, WC, BP], BF16)
            nc.sync.dma_start(
                out=xt, in_=xb[j].rearrange("p (g t) -> p g t", g=WC)
            )
        if j == 3:
            for jj in range(4, NB):
                xt2 = xp.tile([128, WC, BP], BF16, tag="xt2")
                nc.sync.dma_start(
                    out=xt2, in_=xb[jj].rearrange("p (g t) -> p g t", g=WC)
                )
                xts[jj] = xt2
        pj_ps = kqvps.tile([128, 2 * BP], F32, tag="kqvps" + rep)
        kq_ps = pj_ps[:, 0:BP]
        vT_ps = pj_ps[0:64, BP : BP + BP]
        for g in range(WC):
            nc.tensor.matmul(
                kq_ps, wkq_sb[:, ts(g, 128)], xt[:, g, :],
                start=(g == 0), stop=(g == WC - 1),
            )
        blk = slice(j * BP, (j + 1) * BP)
        nc.vector.tensor_scalar_add(kq_sb[:, blk], kq_ps, bkq_sb)
        rm_eng = nc.scalar if j <= 3 else nc.sync
        rm_eng.dma_start(out=q_sb[:, blk], in_=kq_sb[64:128, blk])
        rm_eng.dma_start(out=k2_sb[64:128, blk], in_=kq_sb[0:64, blk])
        for g in range(WC):
            nc.tensor.matmul(
                vT_ps, wv_sb[:, ts(g, H)], xt[:, g, :],
                start=(g == 0), stop=(g == WC - 1),
            )
        nc.vector.tensor_copy(vT_sb[0:64, blk], vT_ps)
        for kk in range(2 * j, 2 * j + 2):
            vp = trps.tile([128, H], BF16, tag="trps" + rep)
            nc.tensor.transpose(
                vp, vT_sb[:, ts(kk, 128)], ident_sb[0:64, 0:64]
            )
            nc.vector.scalar_tensor_tensor(
                v_sb[:, kk, 0:H], vp, 1.0, bvb_sb,
                mybir.AluOpType.mult, mybir.AluOpType.add,
            )
        if probe == "proj":
            continue
        for kind, qidx, kk in POST_SCHED.get(j, ()):
            emit_sched(kind, qidx, kk)

    if probe in ("proj", "nopv"):
        if probe == "nopv":
            for kind, qidx, kk in [("p", 0, 6), ("p", 0, 8), ("p", 0, 10),
                                   ("p", 0, 12), ("p", 0, 14)] +                                   [("p", 1, kk) for kk in range(0, KC, 2)]:
                pass
        nc.sync.dma_start(
            out=out.rearrange("(a p) h -> p a h", p=128),
            in_=kq_sb.bitcast(F32).rearrange("p (a h) -> p a h", h=H),
        )
        return

    # qh1 pairs; qh0's O^T must vacate the single otps buffer before
    # PV(1,0), so its staging copies are emitted (DVE) before that flush
    emit_pair(0, QB, 14)       # last qh0 pair
    emit_pair(QB, QB, 0)       # flushes PV(0,14), PV(0,15)
    emit_pair(QB, QB, 2, flush=False)
    emit_otail_copies(0)       # frees oT_ps[0]
    for kk in range(4, KC, 2):
        emit_pair(QB, QB, kk)
        if kk <= 10:
            i = (kk - 4) // 2
            emit_otail_rest(0, 2 * i, trps)
            emit_otail_rest(0, 2 * i + 1, trps)
    # final pair region-major: all PVs + staging copies first (so the PE
    # never stalls behind a DVE copy), then the per-tile rests
    for h in range(2):
        for q0, qw, k, es in pending:
            nc.tensor.matmul(
                oT_ps[1][:, ts(h, 512)], v_sb[:, k, :], es[:, ts(h, 512)],
                start=False, stop=(k == KC - 1),
            )
        nc.vector.tensor_copy(
            oTs_sb[0 : H + 1, 1, ts(h, 512)], oT_ps[1][:, ts(h, 512)]
        )
    for t in range(QT):
        emit_otail_rest(1, t, scps)
    pending.clear()


def split_multi_waits(nc):
    """This walrus build encodes at most ONE sync-wait per hw instruction.
    Hoist all but the last wait of any multi-wait instruction into standalone
    single-wait NoOps on the same engine queue (semantically identical:
    engine-queue execution is in-order)."""
    import bass_rust

    ctr = 0
    for blk in nc.m.functions[0].blocks:
        insts = blk.instructions
        out = []
        changed = False
        for inst in insts:
            si = inst.sync_info
            if si is not None and si.on_wait and len(si.on_wait) > 1:
                waits = list(si.on_wait)
                for w in waits[:-1]:
                    ctr += 1
                    nop = mybir.InstNoOp(name=f"WSPLIT-{ctr}", ins=[], outs=[])
                    nop.engine = inst.engine
                    nop.sync_info = bass_rust.SyncInfo(on_wait=[w], on_update=[])
                    out.append(nop)
                inst.sync_info = bass_rust.SyncInfo(
                    on_wait=[waits[-1]], on_update=list(si.on_update or [])
                )
                out.append(inst)
                changed = True
            else:
                out.append(inst)
        if changed:
            insts[:] = out
    return nc


def build_bass(split=True, repeat=1, probe=None, **_):
    nc = bass.Bass("TRN2", target_bir_lowering=False, debug=False)
    xbt = nc.dram_tensor("xb", [NB, 128, WC * BP], BF16, kind="ExternalInput").ap()
    wkq = nc.dram_tensor("wkq", [128, WC * 128], BF16, kind="ExternalInput").ap()
    wv = nc.dram_tensor("wv", [128, WC * H], BF16, kind="ExternalInput").ap()
    bkq = nc.dram_tensor("bkq", [128, 1], F32, kind="ExternalInput").ap()
    bv = nc.dram_tensor("bv", [128, H], BF16, kind="ExternalInput").ap()
    identb = nc.dram_tensor("identb", [128, H], BF16, kind="ExternalInput").ap()
    ident = nc.dram_tensor("ident", [128, 128], BF16, kind="ExternalInput").ap()
    mbias = nc.dram_tensor("mbias", [128, KC], F32, kind="ExternalInput").ap()
    vones = nc.dram_tensor("vones", [128, KC], BF16, kind="ExternalInput").ap()
    zpad = nc.dram_tensor("zpad", [128 - H - 1, 2 * QB], BF16,
                          kind="ExternalInput").ap()
    out = nc.dram_tensor("out", [S, H], F32, kind="ExternalOutput").ap()
    with tile.TileContext(nc) as tc:
        for r in range(repeat):
            with ExitStack() as ctx:
                _emit(
                    ctx, tc, xbt, wkq, wv, bkq, bv, identb, ident, mbias,
                    vones, zpad, out, rep=(f"_r{r}" if r else ""), probe=probe,
                )
    if split:
        split_multi_waits(nc)
    return nc


def prep_in_maps(x, attn_mask, Wq, bq, Wk, bk, Wv, bv):
    import ml_dtypes

    bf = ml_dtypes.bfloat16
    x = np.asarray(x, dtype=np.float32)
    attn_mask = np.asarray(attn_mask)
    Wq = np.asarray(Wq, dtype=np.float32)
    Wk = np.asarray(Wk, dtype=np.float32)
    Wv = np.asarray(Wv, dtype=np.float32)
    bq = np.asarray(bq, dtype=np.float32)
    bk = np.asarray(bk, dtype=np.float32)
    bv = np.asarray(bv, dtype=np.float32)

    scale = np.float32(H) ** np.float32(-0.5)
    # [Wk | Wq*scale] -> per-w-chunk stationary layout [128, WC*128]
    wkq = np.concatenate([Wk, Wq * scale], axis=1)  # [W, 128]
    wkq = np.ascontiguousarray(
        wkq.reshape(WC, 128, 128).transpose(1, 0, 2).reshape(128, WC * 128)
    ).astype(bf)
    wv_h = np.ascontiguousarray(
        Wv.reshape(WC, 128, H).transpose(1, 0, 2).reshape(128, WC * H)
    ).astype(bf)
    bkq = np.concatenate([bk, bq * scale]).reshape(128, 1)
    bv_h = np.broadcast_to(bv.reshape(1, H), (128, H)).astype(bf)
    ident = np.eye(128, dtype=np.float32).astype(bf)
    identb = np.ascontiguousarray(
        np.concatenate([np.eye(H), np.eye(H)], axis=0).astype(bf)
    )

    in_maps = []
    for c in range(N_CORES):
        # xb[j, p, g*BP+t] = x[c]^T[g*128+p, j*BP+t]
        xT_c = x[c].T.astype(bf)  # [W, S]
        xb = np.ascontiguousarray(
            xT_c.reshape(WC, 128, NB, BP).transpose(2, 1, 0, 3)
            .reshape(NB, 128, WC * BP)
        )
        m = attn_mask[c].astype(np.float32)  # [S]
        mb = np.where(m != 0, np.float32(0.0), np.float32(-1e30))
        mbias = np.ascontiguousarray(mb.reshape(KC, 128).T)  # [128, KC]
        in_maps.append(
            {
                "xb": xb,
                "wkq": wkq,
                "wv": wv_h,
                "bkq": np.ascontiguousarray(bkq),
                "bv": np.ascontiguousarray(bv_h),
                "identb": identb,
                "ident": ident,
                "mbias": mbias,
                "vones": np.ones((128, KC), dtype=np.float32).astype(bf),
                "zpad": np.zeros((128 - H - 1, 2 * QB), dtype=np.float32).astype(bf),
            }
        )
    return in_maps


def run(x, attn_mask, Wq, bq, Wk, bk, Wv, bv, trace=False, **rb_kwargs):
    from concourse.bass_utils import run_bass_kernel_spmd

    nc = build_bass()
    in_maps = prep_in_maps(x, attn_mask, Wq, bq, Wk, bk, Wv, bv)
    res = run_bass_kernel_spmd(
        nc, in_maps, core_ids=list(range(N_CORES)), trace=trace, **rb_kwargs
    )
    out = np.stack([r["out"] for r in res.results]).astype(np.float32)
    return out, res


def kernel(x, attn_mask, Wq, bq, Wk, bk, Wv, bv):
    out, _ = run(x, attn_mask, Wq, bq, Wk, bk, Wv, bv, trace=False)
    return out


# revision 36
# speedup vs baseline: 2.2057x; 1.0906x over previous
"""Single-head attention on 8 Trainium2 NeuronCores.

Problem: B=8, S=2048, WIDTH=1024, HEAD=64 single attention head.
Sharding: data-parallel over batch -- batch b runs on core b. No collectives.

v2: position-block pipelined, bf16 datapath.

x^T is host-prepped to bf16 and loaded in 8 column blocks of 256
positions (0.5 MB each).  As each block lands, K^T/Q^T (stacked, with
Wq pre-scaled) and V^T are projected for those positions, copied out
with biases on DVE, partition-remapped (Q^T down to 0:64, K^T up to
64:128) via DMA, and V^T chunks PE-transposed into V' (+ones column).
Attention units (k-chunk x query-half) are emitted interleaved with the
block loop as soon as their inputs exist, so the ACT exp chain (the
critical 33us of work) starts ~4us in, overlapping the remaining x DMA
and projections instead of following them.

Per unit (qh, k): scores^T = K^T_k.T @ Q^T[qh] ([128,1024] PSUM, even k
on PE row-half A, odd k row-half B via tile_position -- concurrent);
es = exp(scores^T + mask_bias) on ACT (bf16 out); O'^T[qh] += V'_k.T @ es
(PE, accumulating [65, 1024] PSUM; row 64 = softmax denominators from
the ones column).  O-tail per 128-query tile: bf16 staging copy,
PE-transpose, DVE reciprocal + scale, DMA out.
"""

import os
from contextlib import ExitStack

import numpy as np

import concourse.bass as bass
import concourse.tile as tile
from concourse import mybir
from concourse.bass import ts

S = 2048
W = 1024
H = 64
N_CORES = 8
WC = W // 128   # 8 w-chunks
KC = S // 128   # 16 k-chunks
NB = 8          # x position blocks
BP = S // NB    # 256 positions per block
QB = 1024       # query-half size
QT = QB // 128  # 8 q-tiles per half

F32 = mybir.dt.float32
BF16 = mybir.dt.bfloat16
AF = mybir.ActivationFunctionType

# score/exp/PV work is emitted as pairs (k-chunks kk, kk+1 on PE row
# halves A/B, back-to-back so they overlap) over a query range.  Early
# pairs use 512-wide query granules (qg = index in units of 512) so the
# ACT exp chain starts ~4.5us in, while x blocks 2..7 are still loading;
# once queries 0:1024 exist, pairs go 1024-wide (qh units).
# PRE pairs depend only on OLDER blocks and are emitted before the
# section's projection matmuls, so the exp chain never waits on proj.
PRE_SCHED = {
    1: [("h", 0, 0)],
    2: [("g", 0, 2)],
    4: [("p", 0, 6)],
    5: [("p", 0, 8)],
    6: [("p", 0, 10)],
    7: [("p", 0, 12)],
}
POST_SCHED = {
    1: [("h", 1, 0)],
    2: [("g", 0, 4)],
    3: [("g", 1, 0), ("g", 1, 2), ("g", 1, 4)],
}


def _emit(ctx, tc, xb, wkq, wv, bkq, bv, identb, ident, mbias, vones, zpad,
          out, rep="", probe=None):
    nc = tc.nc

    def pool(name, **kw):
        return ctx.enter_context(tc.tile_pool(name=name + rep, **kw))

    singles = pool("singles", bufs=1)
    wkq_sb = singles.tile([128, WC * 128], BF16)
    nc.scalar.dma_start(out=wkq_sb, in_=wkq)
    wv_sb = singles.tile([128, WC * H], BF16)
    nc.scalar.dma_start(out=wv_sb, in_=wv)
    bkq_sb = singles.tile([128, 1], F32)
    nc.scalar.dma_start(out=bkq_sb, in_=bkq)
    bvb_sb = singles.tile([128, H], BF16)
    nc.scalar.dma_start(out=bvb_sb, in_=bv)
    identb_sb = singles.tile([128, H], BF16)
    nc.scalar.dma_start(out=identb_sb, in_=identb)
    ident_sb = singles.tile([128, 128], BF16)
    nc.scalar.dma_start(out=ident_sb, in_=ident)
    mbias_sb = singles.tile([128, KC], F32)
    nc.scalar.dma_start(out=mbias_sb, in_=mbias)
    warm_sb = singles.tile([1, 1], F32)
    nc.scalar.activation(warm_sb, mbias_sb[0:1, 0:1], AF.Exp,
                         bias=mbias_sb[0:1, 0:1], scale=1.0)

    kq_sb = singles.tile([128, S], BF16)   # rows 0:64 K^T, rows 64:128 Q^T
    q_sb = singles.tile([64, S], BF16)     # Q^T at partitions 0:64
    k2_sb = singles.tile([128, S], BF16)   # K^T replicated at partitions 64:128
    vT_sb = singles.tile([64, S], BF16)
    v_sb = singles.tile([128, KC, H + 1], BF16)  # V' chunks (+ones col)
    oTs_sb = singles.tile([128, 2, QB], BF16)    # O^T staging, rows 65:128 zero
    nc.gpsimd.dma_start(
        out=v_sb[:, :, H : H + 1],
        in_=vones.rearrange("p (k one) -> p k one", one=1),
    )
    nc.gpsimd.dma_start(
        out=oTs_sb[H + 1 : 128, :, :],
        in_=zpad.rearrange("p (a b) -> p a b", a=2),
    )

    # pools
    xp = pool("xp", bufs=3)
    kqvps = pool("kqvps", bufs=1, space="PSUM")
    scps = pool("scps", bufs=2, space="PSUM")
    otps = pool("otps", bufs=1, space="PSUM")
    trps = pool("trps", bufs=1, space="PSUM")
    es_pool = pool("es", bufs=6)
    rec_pool = pool("rec", bufs=4)
    ob_pool = pool("ob", bufs=2)

    out_v = out.rearrange("(qh t p) h -> qh p t h", p=128, t=QT)

    # PE clock warm-up: the HAM clock-gate holds the PE at 1.2 GHz until it
    # has seen ~3us of sustained activity, which would otherwise tax the
    # startup-critical first projections.  The PE is idle until x block 0
    # lands (~2.9us), so spend that window on discarded matmuls over a
    # zeroed tile to open the gate before proj(0) issues.
    wu_sb = singles.tile([128, 512], BF16)
    nc.vector.memset(wu_sb, 0.0)
    for _ in range(6):
        wu_ps = scps.tile([128, 512], F32, tag="scps" + rep)
        nc.tensor.matmul(wu_ps, wu_sb[:, 0:128], wu_sb)

    oT_ps = [None, None]
    ob_sb = [None, None]
    pending = []   # (q0, qw, k, es) awaiting their PV matmul
    started = set()  # (qh, h) oT 512-regions already start=True'd

    def flush_pv():
        for q0, qw, k, es in pending:
            qh = q0 // QB
            if oT_ps[qh] is None:
                ot_tile = otps.tile([H + 1, QB], F32, tag="otps" + rep)
                oT_ps[qh] = ot_tile
            for h in range(qw // 512):
                hreg = (q0 % QB) // 512 + h
                st = (qh, hreg) not in started
                started.add((qh, hreg))
                nc.tensor.matmul(
                    oT_ps[qh][:, ts(hreg, 512)], v_sb[:, k, :],
                    es[:, ts(h, 512)], start=st, stop=(k == KC - 1),
                )
        pending.clear()

    def emit_pair(q0, qw, kk, flush=True):
        sc0 = scps.tile([128, qw], F32, tag="scps" + rep)
        sc1 = scps.tile([128, qw], F32, tag="scps" + rep)
        step = min(qw, 512)
        for h in range(qw // step):
            hs = slice(q0 + h * step, q0 + (h + 1) * step)
            nc.tensor.matmul(sc0[:, ts(h, step)], kq_sb[0:64, ts(kk, 128)],
                             q_sb[:, hs])
            nc.tensor.matmul(
                sc1[:, ts(h, step)], k2_sb[64:128, ts(kk + 1, 128)],
                kq_sb[64:128, hs], tile_position=(64, 0),
            )
        if flush and probe != "nopv":
            flush_pv()
        for k, sc in ((kk, sc0), (kk + 1, sc1)):
            es = es_pool.tile([128, qw], BF16)
            nc.scalar.activation(
                es, sc, AF.Exp, bias=mbias_sb[:, k : k + 1], scale=1.0
            )
            if probe != "nopv":
                pending.append((q0, qw, k, es))

    h_es = {}

    def emit_half_pair(half, kk):
        # scores+exp for k-chunks kk,kk+1 on queries half*256:(half+1)*256,
        # filling half of a shared 512-wide es tile; PV deferred to half 1
        hs = slice(half * 256, (half + 1) * 256)
        scs = []
        for i, k in enumerate((kk, kk + 1)):
            sc = scps.tile([128, 256], F32, tag="scps" + rep)
            if k % 2 == 1:
                nc.tensor.matmul(
                    sc, k2_sb[64:128, ts(k, 128)], kq_sb[64:128, hs],
                    tile_position=(64, 0),
                )
            else:
                nc.tensor.matmul(sc, kq_sb[0:64, ts(k, 128)], q_sb[:, hs])
            scs.append(sc)
        for i, k in enumerate((kk, kk + 1)):
            if (k, "es") not in h_es:
                es_h = es_pool.tile([128, 512], BF16, tag="esh")
                h_es[(k, "es")] = es_h
            es = h_es[(k, "es")]
            nc.scalar.activation(
                es[:, hs], scs[i], AF.Exp,
                bias=mbias_sb[:, k : k + 1], scale=1.0,
            )
            if half == 1:
                pending.append((0, 512, k, es))

    def emit_sched(kind, qidx, kk):
        if kind == "h":
            emit_half_pair(qidx, kk)
        elif kind == "g":
            emit_pair(qidx * 512, 512, kk)
        else:
            emit_pair(qidx * QB, QB, kk)

    def emit_otail_copies(qh):
        for h in range(2):
            nc.vector.tensor_copy(
                oTs_sb[0 : H + 1, qh, ts(h, 512)], oT_ps[qh][:, ts(h, 512)]
            )

    def emit_otail_rest(qh, t, ps_pool):
        if ob_sb[qh] is None:
            ob_tile = ob_pool.tile([128, QT, H], F32, tag="ob" + rep)
            ob_sb[qh] = ob_tile
        op = ps_pool.tile([128, 128], BF16, tag=ps_pool.name)
        nc.tensor.transpose(op, oTs_sb[:, qh, ts(t, 128)], ident_sb)
        rec = rec_pool.tile([128, 1], F32)
        nc.vector.reciprocal(rec, op[:, H : H + 1])
        nc.vector.tensor_scalar_mul(ob_sb[qh][:, t, :], op[:, 0:H], rec)
        if t % 4 == 3:
            dma_eng = nc.scalar if (qh == 1 and t == QT - 1) else nc.sync
            dma_eng.dma_start(
                out=out_v[qh][:, t - 3 : t + 1, :],
                in_=ob_sb[qh][:, t - 3 : t + 1, :],
            )

    # ---------------- pipelined block loop ----------------
    xts = {}
    for j in range(NB):
        for kind, qidx, kk in PRE_SCHED.get(j, ()) if probe != "proj" else ():
            emit_sched(kind, qidx, kk)
        if j in xts:
            xt = xts.pop(j)
        else:
            xt = xp.tile([128, WC, BP], BF16)
            nc.sync.dma_start(
                out=xt, in_=xb[j].rearrange("p (g t) -> p g t", g=WC)
            )
        if j == 3:
            for jj in range(4, NB):
                xt2 = xp.tile([128, WC, BP], BF16, tag="xt2")
                nc.sync.dma_start(
                    out=xt2, in_=xb[jj].rearrange("p (g t) -> p g t", g=WC)
                )
                xts[jj] = xt2
        pj_ps = kqvps.tile([128, 2 * BP], F32, tag="kqvps" + rep)
        kq_ps = pj_ps[:, 0:BP]
        vT_ps = pj_ps[0:64, BP : BP + BP]
        for g in range(WC):
            nc.tensor.matmul(
                kq_ps, wkq_sb[:, ts(g, 128)], xt[:, g, :],
                start=(g == 0), stop=(g == WC - 1),
            )
        blk = slice(j * BP, (j + 1) * BP)
        nc.vector.tensor_scalar_add(kq_sb[:, blk], kq_ps, bkq_sb)
        rm_eng = nc.scalar if j <= 3 else nc.sync
        rm_eng.dma_start(out=q_sb[:, blk], in_=kq_sb[64:128, blk])
        rm_eng.dma_start(out=k2_sb[64:128, blk], in_=kq_sb[0:64, blk])
        for g in range(WC):
            nc.tensor.matmul(
                vT_ps, wv_sb[:, ts(g, H)], xt[:, g, :],
                start=(g == 0), stop=(g == WC - 1),
            )
        nc.vector.tensor_copy(vT_sb[0:64, blk], vT_ps)
        for kk in range(2 * j, 2 * j + 2):
            vp = trps.tile([128, H], BF16, tag="trps" + rep)
            nc.tensor.transpose(
                vp, vT_sb[:, ts(kk, 128)], ident_sb[0:64, 0:64]
            )
            nc.vector.scalar_tensor_tensor(
                v_sb[:, kk, 0:H], vp, 1.0, bvb_sb,
                mybir.AluOpType.mult, mybir.AluOpType.add,
            )
        if probe == "proj":
            continue
        for kind, qidx, kk in POST_SCHED.get(j, ()):
            emit_sched(kind, qidx, kk)

    if probe in ("proj", "nopv"):
        if probe == "nopv":
            for kind, qidx, kk in [("p", 0, 6), ("p", 0, 8), ("p", 0, 10),
                                   ("p", 0, 12), ("p", 0, 14)] +                                   [("p", 1, kk) for kk in range(0, KC, 2)]:
                pass
        nc.sync.dma_start(
            out=out.rearrange("(a p) h -> p a h", p=128),
            in_=kq_sb.bitcast(F32).rearrange("p (a h) -> p a h", h=H),
        )
        return

    # qh1 pairs; qh0's O^T must vacate the single otps buffer before
    # PV(1,0), so its staging copies are emitted (DVE) before that flush
    emit_pair(0, QB, 14)       # last qh0 pair
    emit_pair(QB, QB, 0)       # flushes PV(0,14), PV(0,15)
    emit_pair(QB, QB, 2, flush=False)
    emit_otail_copies(0)       # frees oT_ps[0]
    for kk in range(4, KC, 2):
        emit_pair(QB, QB, kk)
        if kk <= 10:
            i = (kk - 4) // 2
            emit_otail_rest(0, 2 * i, trps)
            emit_otail_rest(0, 2 * i + 1, trps)
    # final pair region-major: all PVs + staging copies first (so the PE
    # never stalls behind a DVE copy), then the per-tile rests
    for h in range(2):
        for q0, qw, k, es in pending:
            nc.tensor.matmul(
                oT_ps[1][:, ts(h, 512)], v_sb[:, k, :], es[:, ts(h, 512)],
                start=False, stop=(k == KC - 1),
            )
        nc.vector.tensor_copy(
            oTs_sb[0 : H + 1, 1, ts(h, 512)], oT_ps[1][:, ts(h, 512)]
        )
    for t in range(QT):
        emit_otail_rest(1, t, scps)
    pending.clear()


def split_multi_waits(nc):
    """This walrus build encodes at most ONE sync-wait per hw instruction.
    Hoist all but the last wait of any multi-wait instruction into standalone
    single-wait NoOps on the same engine queue (semantically identical:
    engine-queue execution is in-order)."""
    import bass_rust

    ctr = 0
    for blk in nc.m.functions[0].blocks:
        insts = blk.instructions
        out = []
        changed = False
        for inst in insts:
            si = inst.sync_info
            if si is not None and si.on_wait and len(si.on_wait) > 1:
                waits = list(si.on_wait)
                for w in waits[:-1]:
                    ctr += 1
                    nop = mybir.InstNoOp(name=f"WSPLIT-{ctr}", ins=[], outs=[])
                    nop.engine = inst.engine
                    nop.sync_info = bass_rust.SyncInfo(on_wait=[w], on_update=[])
                    out.append(nop)
                inst.sync_info = bass_rust.SyncInfo(
                    on_wait=[waits[-1]], on_update=list(si.on_update or [])
                )
                out.append(inst)
                changed = True
            else:
                out.append(inst)
        if changed:
            insts[:] = out
    return nc


def build_bass(split=True, repeat=1, probe=None, **_):
    nc = bass.Bass("TRN2", target_bir_lowering=False, debug=False)
    xbt = nc.dram_tensor("xb", [NB, 128, WC * BP], BF16, kind="ExternalInput").ap()
    wkq = nc.dram_tensor("wkq", [128, WC * 128], BF16, kind="ExternalInput").ap()
    wv = nc.dram_tensor("wv", [128, WC * H], BF16, kind="ExternalInput").ap()
    bkq = nc.dram_tensor("bkq", [128, 1], F32, kind="ExternalInput").ap()
    bv = nc.dram_tensor("bv", [128, H], BF16, kind="ExternalInput").ap()
    identb = nc.dram_tensor("identb", [128, H], BF16, kind="ExternalInput").ap()
    ident = nc.dram_tensor("ident", [128, 128], BF16, kind="ExternalInput").ap()
    mbias = nc.dram_tensor("mbias", [128, KC], F32, kind="ExternalInput").ap()
    vones = nc.dram_tensor("vones", [128, KC], BF16, kind="ExternalInput").ap()
    zpad = nc.dram_tensor("zpad", [128 - H - 1, 2 * QB], BF16,
                          kind="ExternalInput").ap()
    out = nc.dram_tensor("out", [S, H], F32, kind="ExternalOutput").ap()
    with tile.TileContext(nc) as tc:
        for r in range(repeat):
            with ExitStack() as ctx:
                _emit(
                    ctx, tc, xbt, wkq, wv, bkq, bv, identb, ident, mbias,
                    vones, zpad, out, rep=(f"_r{r}" if r else ""), probe=probe,
                )
    if split:
        split_multi_waits(nc)
    return nc


def prep_in_maps(x, attn_mask, Wq, bq, Wk, bk, Wv, bv):
    import ml_dtypes

    bf = ml_dtypes.bfloat16
    x = np.asarray(x, dtype=np.float32)
    attn_mask = np.asarray(attn_mask)
    Wq = np.asarray(Wq, dtype=np.float32)
    Wk = np.asarray(Wk, dtype=np.float32)
    Wv = np.asarray(Wv, dtype=np.float32)
    bq = np.asarray(bq, dtype=np.float32)
    bk = np.asarray(bk, dtype=np.float32)
    bv = np.asarray(bv, dtype=np.float32)

    scale = np.float32(H) ** np.float32(-0.5)
    # [Wk | Wq*scale] -> per-w-chunk stationary layout [128, WC*128]
    wkq = np.concatenate([Wk, Wq * scale], axis=1)  # [W, 128]
    wkq = np.ascontiguousarray(
        wkq.reshape(WC, 128, 128).transpose(1, 0, 2).reshape(128, WC * 128)
    ).astype(bf)
    wv_h = np.ascontiguousarray(
        Wv.reshape(WC, 128, H).transpose(1, 0, 2).reshape(128, WC * H)
    ).astype(bf)
    bkq = np.concatenate([bk, bq * scale]).reshape(128, 1)
    bv_h = np.broadcast_to(bv.reshape(1, H), (128, H)).astype(bf)
    ident = np.eye(128, dtype=np.float32).astype(bf)
    identb = np.ascontiguousarray(
        np.concatenate([np.eye(H), np.eye(H)], axis=0).astype(bf)
    )

    in_maps = []
    for c in range(N_CORES):
        # xb[j, p, g*BP+t] = x[c]^T[g*128+p, j*BP+t]
        xT_c = x[c].T.astype(bf)  # [W, S]
        xb = np.ascontiguousarray(
            xT_c.reshape(WC, 128, NB, BP).transpose(2, 1, 0, 3)
            .reshape(NB, 128, WC * BP)
        )
        m = attn_mask[c].astype(np.float32)  # [S]
        mb = np.where(m != 0, np.float32(0.0), np.float32(-1e30))
        mbias = np.ascontiguousarray(mb.reshape(KC, 128).T)  # [128, KC]
        in_maps.append(
            {
                "xb": xb,
                "wkq": wkq,
                "wv": wv_h,
                "bkq": np.ascontiguousarray(bkq),
                "bv": np.ascontiguousarray(bv_h),
                "identb": identb,
                "ident": ident,
                "mbias": mbias,
                "vones": np.ones((128, KC), dtype=np.float32).astype(bf),
                "zpad": np.zeros((128 - H - 1, 2 * QB), dtype=np.float32).astype(bf),
            }
        )
    return in_maps


def run(x, attn_mask, Wq, bq, Wk, bk, Wv, bv, trace=False, **rb_kwargs):
    from concourse.bass_utils import run_bass_kernel_spmd

    nc = build_bass()
    in_maps = prep_in_maps(x, attn_mask, Wq, bq, Wk, bk, Wv, bv)
    res = run_bass_kernel_spmd(
        nc, in_maps, core_ids=list(range(N_CORES)), trace=trace, **rb_kwargs
    )
    out = np.stack([r["out"] for r in res.results]).astype(np.float32)
    return out, res


def kernel(x, attn_mask, Wq, bq, Wk, bk, Wv, bv):
    out, _ = run(x, attn_mask, Wq, bq, Wk, bk, Wv, bv, trace=False)
    return out
